# revision 22
# baseline (speedup 1.0000x reference)
"""Trainium2 Bass kernel for nn_BasicRecurrentEntityEncoder.

Data-parallel over batch B=256 across 8 NeuronCores (32 batches/core).
Per core, entity rows are laid out k-major: j = k*32 + b (K padded 30->32),
giving 1024 rows = 8 chunks x 128 partitions with b = p % 32 uniform in
every chunk. State is kept both natural ([128, 8, 256], row-major) and
transposed ([d, j], for PE matmuls); the transpose is refreshed on PE each
step via identity matmuls.

Phase A: indirect-DMA gather of bf16 embedding rows (masked tokens point at
a zero row), on-chip bag-of-words sums, encT / e@W / e.keys (EK) / keys@V
precomputes. Phase B: the 64-step recurrence, split into two 4-chunk groups
per step so PE/ACT/DVE work on different groups concurrently. The gate
sigmoid runs as exp (ACT) + reciprocal (DVE); rsqrt is a magic-seed Newton
iteration on DVE, so the only ACT table set ever needed is exp_and_others.
"""

import os
import numpy as np
import ml_dtypes

B, S, L, D, K, VOCAB = 256, 64, 16, 256, 30, 50000
NCORES = 8
BL = B // NCORES          # 32 batches per core
KH = 32                   # padded K
J = KH * BL               # 1024 rows per core
CH = 8                    # row chunks (128 partitions each)
NG = 2                    # chunk groups per step
CPG = CH // NG            # chunks per group
GRP = 8                   # gather groups
SPG = S // GRP            # steps per group
VPAD = VOCAB + 8          # embedding rows incl. zero pad rows
NEG = -60.0               # gate logit offset for masked sentences
EPS = 1e-12

LAST_EXEC_NS = None       # set when BASS_KERNEL_TRACE=1
NSTEPS = int(os.environ.get("BK_NSTEPS", str(S)))
SKIP_GATHER = os.environ.get("BK_SKIP_GATHER", "0") == "1"

_bf16 = ml_dtypes.bfloat16


def _build_nc():
    import concourse.bacc as bacc
    import concourse.bass as bass
    import concourse.mybir as mybir
    from concourse import tile

    f32 = mybir.dt.float32
    bf16 = mybir.dt.bfloat16
    i32 = mybir.dt.int32
    MULT = mybir.AluOpType.mult
    ADD = mybir.AluOpType.add

    nc = bacc.Bacc("TRN2", target_bir_lowering=False, debug=False,
                   num_devices=NCORES)

    # ---- DRAM parameters -------------------------------------------------
    emb = nc.dram_tensor("emb", [8192, 4 * D], bf16, kind="ExternalInput")
    idx_d = nc.dram_tensor("idx", [128, 512], mybir.dt.int16, kind="ExternalInput")
    keysT_d = nc.dram_tensor("keysT", [128, 2, J], bf16, kind="ExternalInput")
    u_d = nc.dram_tensor("u", [128, 2, D], bf16, kind="ExternalInput")
    v_d = nc.dram_tensor("v", [128, 2, D], bf16, kind="ExternalInput")
    w_d = nc.dram_tensor("w", [128, 2, D], bf16, kind="ExternalInput")
    bias_d = nc.dram_tensor("bias", [128, S], f32, kind="ExternalInput")
    selsum_d = nc.dram_tensor("selsum", [128, BL], bf16, kind="ExternalInput")
    selkm_d = nc.dram_tensor("selkm", [BL, 128], bf16, kind="ExternalInput")
    mdiag_d = nc.dram_tensor("mdiag", [128, BL], f32, kind="ExternalInput")
    ident_d = nc.dram_tensor("ident", [128, 128], bf16, kind="ExternalInput")
    y_d = nc.dram_tensor("y", [BL, K, D], f32, kind="ExternalOutput")

    with tile.TileContext(nc) as tc:
        ctxs = []

        def pool(name, bufs, space="SBUF"):
            p = tc.tile_pool(name=name, bufs=bufs, space=space)
            ctxs.append(p)
            return p.__enter__()

        persist = pool("persist", 1)
        gbuf = pool("gbuf", 2)
        work = pool("work", 2)                  # per-(step, group) scratch
        ps_pn = pool("ps_pn", 2, "PSUM")        # [128, 4, 256] f32 = 2 banks
        ps_sm = pool("ps_sm", 2, "PSUM")        # [128, 256] f32 slots
        ps_t = pool("ps_t", 2, "PSUM")          # [128, 2, 512] bf16 = 1 bank

        # ---- persistent SBUF tensors ------------------------------------
        idx_sb = persist.tile([128, 512], mybir.dt.int16, tag="idx")
        keysT = persist.tile([128, 2, J], bf16, tag="keysT")
        u_sb = persist.tile([128, 2, D], bf16, tag="u")
        v_sb = persist.tile([128, 2, D], bf16, tag="v")
        w_sb = persist.tile([128, 2, D], bf16, tag="w")
        bias_sb = persist.tile([128, S], f32, tag="bias")
        selsum = persist.tile([128, BL], bf16, tag="selsum")
        selkm = persist.tile([BL, 128], bf16, tag="selkm")
        mdiag = persist.tile([128, BL], f32, tag="mdiag")
        ident = persist.tile([128, 128], bf16, tag="ident")
        encT = persist.tile([128, 2, S * BL], bf16, tag="encT")
        ew_all = persist.tile([BL, S * D], bf16, tag="ew")
        ekm = persist.tile([128, CH, S], f32, tag="ekm")
        kv = persist.tile([128, CH, D], bf16, tag="kv")
        h_nat = persist.tile([128, CH, D], bf16, tag="h_nat")
        hT = persist.tile([128, 2, J], bf16, tag="hT")
        hf32 = persist.tile([128, CH, D], f32, tag="hf32")

        # ---- load parameters --------------------------------------------
        nc.sync.dma_start(out=idx_sb[:], in_=idx_d.ap())
        nc.sync.dma_start(out=keysT[:], in_=keysT_d.ap())
        nc.sync.dma_start(out=u_sb[:], in_=u_d.ap())
        nc.sync.dma_start(out=v_sb[:], in_=v_d.ap())
        nc.sync.dma_start(out=w_sb[:], in_=w_d.ap())
        nc.sync.dma_start(out=bias_sb[:], in_=bias_d.ap())
        nc.sync.dma_start(out=selsum[:], in_=selsum_d.ap())
        nc.sync.dma_start(out=selkm[:], in_=selkm_d.ap())
        nc.sync.dma_start(out=mdiag[:], in_=mdiag_d.ap())
        nc.sync.dma_start(out=ident[:], in_=ident_d.ap())

        nc.vector.memset(h_nat[:], 0.0)
        nc.vector.memset(hT[:], 0.0)

        # ========== interleaved: gathers + per-group precompute + scan ====
        def emit_gather(g):
            raw = gbuf.tile([128, 4 * SPG, D], bf16, tag="raw")
            nc.gpsimd.dma_gather(
                out_ap=raw[:].rearrange("p (q k) d -> p q (k d)", k=4),
                in_ap=emb.ap(),
                idxs_ap=idx_sb[:, g * 64:(g + 1) * 64],
                num_idxs=1024, num_idxs_reg=1024, elem_size=4 * D)
            return raw

        def emit_group_precompute(g, raw):
            # l-sum: raw[p, (s_in, l_hi), d] -> part[p, s_in, d]
            s02 = gbuf.tile([128, SPG, 2, D], bf16, tag="s02")
            r4 = raw[:].rearrange("p (s l) d -> p s l d", l=4)
            nc.vector.tensor_tensor(out=s02[:], in0=r4[:, :, 0:2, :],
                                    in1=r4[:, :, 2:4, :], op=ADD)
            part = gbuf.tile([128, SPG, D], bf16, tag="part")
            nc.vector.tensor_tensor(out=part[:], in0=s02[:, :, 0, :],
                                    in1=s02[:, :, 1, :], op=ADD)
            # encT[half][d, (s, b)] via PE: part.T @ selsum
            for half in range(2):
                etp = ps_sm.tile([128, SPG * BL], f32, tag="sm")
                for si in range(SPG):
                    nc.tensor.matmul(
                        out=etp[:, si * BL:(si + 1) * BL],
                        lhsT=part[:, si, half * 128:(half + 1) * 128],
                        rhs=selsum[:], start=(si == 0), stop=(si == SPG - 1))
                nc.vector.tensor_copy(
                    out=encT[:, half, g * SPG * BL:(g + 1) * SPG * BL],
                    in_=etp[:])
            # eW[b, (s, d)] for this group, in two 4-step halves
            for hg in range(2):
                ewp = ps_pn.tile([BL, 4, D], f32, tag="pn")
                for si in range(4):
                    s = g * SPG + hg * 4 + si
                    for half in range(2):
                        nc.tensor.matmul(
                            out=ewp[:, si, :],
                            lhsT=encT[:, half, s * BL:(s + 1) * BL],
                            rhs=w_sb[:, half, :],
                            start=(half == 0 and si % 2 == 0),
                            stop=(half == 1 and si % 2 == 1))
                nc.vector.tensor_copy(
                    out=ew_all[:, (g * SPG + hg * 4) * D:
                               (g * SPG + hg * 4 + 4) * D],
                    in_=ewp[:])
            # EK for this group -> ekm[:, :, 8g:8g+8], in two 4-chunk halves
            for cg in range(2):
                gbig = ps_pn.tile([128, 4, SPG * BL], f32, tag="pn")
                for ci in range(4):
                    c = cg * 4 + ci
                    for half in range(2):
                        nc.tensor.matmul(
                            out=gbig[:, ci, :],
                            lhsT=keysT[:, half, c * 128:(c + 1) * 128],
                            rhs=encT[:, half, g * SPG * BL:(g + 1) * SPG * BL],
                            start=(half == 0 and ci % 2 == 0),
                            stop=(half == 1 and ci % 2 == 1))
                eks = work.tile([128, 4, SPG, BL], f32, tag="ekscr")
                nc.vector.tensor_tensor(
                    out=eks[:],
                    in0=gbig[:].rearrange("p c (s b) -> p c s b", s=SPG),
                    in1=mdiag[:].unsqueeze(1).unsqueeze(1).broadcast_to(
                        [128, 4, SPG, BL]),
                    op=MULT)
                red = work.tile([128, 4, SPG], f32, tag="ekred")
                nc.vector.tensor_reduce(
                    out=red[:], in_=eks[:], axis=mybir.AxisListType.X, op=ADD)
                nc.vector.tensor_tensor(
                    out=ekm[:, cg * 4:(cg + 1) * 4, g * SPG:(g + 1) * SPG],
                    in0=red[:],
                    in1=bias_sb[:, g * SPG:(g + 1) * SPG].unsqueeze(1)
                    .broadcast_to([128, 4, SPG]),
                    op=ADD)

        # kV[p, c, d] = keys @ V (needs only keysT)
        for c in range(CH):
            kvp = ps_sm.tile([128, D], f32, tag="sm")
            for half in range(2):
                nc.tensor.matmul(out=kvp[:],
                                 lhsT=keysT[:, half, c * 128:(c + 1) * 128],
                                 rhs=v_sb[:, half, :],
                                 start=(half == 0), stop=(half == 1))
            nc.vector.tensor_copy(out=kv[:, c, :], in_=kvp[:])

        EXP = mybir.ActivationFunctionType.Exp
        RELU = mybir.ActivationFunctionType.Relu
        SQUARE = mybir.ActivationFunctionType.Square
        COPYF = mybir.ActivationFunctionType.Copy
        TANH = mybir.ActivationFunctionType.Tanh

        def scan_step(s):
            last = (s == NSTEPS - 1)
            # --- PSUM tiles: one gate tile per step, pn per 4-chunk group
            gps = ps_sm.tile([128, CH * BL], f32, tag="sm")
            pns = []
            # gate mms first: frees hT earliest, gate chain overlaps hU
            for c in range(CH):
                for half in range(2):
                    nc.tensor.matmul(out=gps[:, c * BL:(c + 1) * BL],
                                     lhsT=hT[:, half, c * 128:(c + 1) * 128],
                                     rhs=encT[:, half, s * BL:(s + 1) * BL],
                                     start=(c == 0 and half == 0),
                                     stop=(c == CH - 1 and half == 1))
            for G in range(NG):
                c0 = G * CPG
                pn = ps_pn.tile([128, CPG, D], f32, tag="pn")
                pns.append(pn)
                for i in range(2):
                    nc.tensor.matmul(out=pn[:, 2 * i:2 * i + 2, :],
                                     lhsT=ident[:],
                                     rhs=kv[:, c0 + 2 * i:c0 + 2 * i + 2, :],
                                     start=True, stop=False)
                ews = ew_all[:, s * D:(s + 1) * D]
                for i in range(2):
                    nc.tensor.matmul(
                        out=pn[:, 2 * i:2 * i + 2, :], lhsT=selkm[:],
                        rhs=ews.unsqueeze(1).broadcast_to([BL, 2, D]),
                        start=False, stop=False)
                for ci in range(CPG):
                    c = c0 + ci
                    for half in range(2):
                        nc.tensor.matmul(out=pn[:, ci, :],
                                         lhsT=hT[:, half,
                                                 c * 128:(c + 1) * 128],
                                         rhs=u_sb[:, half, :], start=False,
                                         stop=(half == 1 and ci % 2 == 1))
            # --- gate vec chain + relus, interleaved so the TANH is not
            # stuck behind all four relu pairs on the ACT queue ------------
            h_tld = work.tile([128, CH, D], bf16, tag="h_tld")
            gm = work.tile([128, CH, BL], f32, tag="gm")
            nc.vector.tensor_tensor(
                out=gm[:], in0=gps[:].rearrange("p (c b) -> p c b", b=BL),
                in1=mdiag[:].unsqueeze(1).broadcast_to([128, CH, BL]),
                op=MULT)
            vec = work.tile([128, 4, CH], f32, tag="vec")
            gpre, g_all = vec[:, 0, :], vec[:, 1, :]
            ss, ny0 = vec[:, 2, :], vec[:, 3, :]
            nc.vector.tensor_reduce(out=gpre, in_=gm[:],
                                    axis=mybir.AxisListType.X, op=ADD)
            nc.vector.tensor_tensor(out=gpre, in0=gpre,
                                    in1=ekm[:, :, s], op=ADD)

            def relu_pair(G, i):
                c0 = G * CPG
                nc.scalar.activation(
                    h_tld[:, c0 + 2 * i:c0 + 2 * i + 2, :]
                    .rearrange("p c d -> p (c d)"),
                    pns[G][:, 2 * i:2 * i + 2, :]
                    .rearrange("p c d -> p (c d)"), RELU)

            relu_pair(0, 0)
            # g = sigmoid(x) = 0.5 + 0.5 * tanh(x / 2)  (tanh is in the
            # exp_and_others set; saves the reciprocal on the gate chain)
            nc.scalar.activation(gpre, gpre, TANH, scale=0.5)
            nc.vector.tensor_scalar(out=g_all, in0=gpre, scalar1=0.5,
                                    scalar2=0.5, op0=MULT, op1=ADD)
            relu_pair(0, 1)
            relu_pair(1, 0)
            relu_pair(1, 1)
            # --- upd = g*h_tld + h_nat (DVE); ss = ||upd||^2 (ACT) -------
            upd = work.tile([128, CH, D], bf16, tag="upd")
            sqd = work.tile([128, CH, D], bf16, tag="sqd")
            for c in range(CH):
                nc.vector.scalar_tensor_tensor(
                    out=upd[:, c, :], in0=h_tld[:, c, :],
                    scalar=g_all[:, c:c + 1], in1=h_nat[:, c, :],
                    op0=MULT, op1=ADD)
            for c in range(CH):
                if c % 3 == 2:        # chunks 2, 5 -> DVE; rest -> ACT
                    nc.vector.scalar_tensor_tensor(
                        out=sqd[:, c, :], in0=upd[:, c, :], scalar=1.0,
                        in1=upd[:, c, :], op0=MULT, op1=MULT,
                        accum_out=ss[:, c:c + 1])
                else:
                    nc.scalar.activation(sqd[:, c, :], upd[:, c, :], SQUARE,
                                         accum_out=ss[:, c:c + 1])
            # --- per-group: r = rsqrt(ss) (magic seed + one Newton step,
            # zero-safe), normalize, transpose refresh. Per-group tails let
            # G0's copies and next-step matmuls start before G1 finishes. -
            nw_ = vec[:, 0, :]          # gpre slot is dead now
            for G in range(NG):
                c0 = G * CPG
                sl = slice(c0, c0 + CPG)
                ssG, nyG, nwG = ss[:, sl], ny0[:, sl], nw_[:, sl]
                nc.vector.tensor_scalar(
                    out=nyG.bitcast(i32), in0=ssG.bitcast(i32), scalar1=1,
                    scalar2=-1, op0=mybir.AluOpType.logical_shift_right,
                    op1=mybir.AluOpType.bitwise_xor)
                nc.vector.tensor_scalar(
                    out=nyG.bitcast(i32), in0=nyG.bitcast(i32),
                    scalar1=0x5f3759e0, scalar2=None, op0=ADD)
                nc.vector.tensor_tensor(out=nwG, in0=ssG, in1=nyG, op=MULT)
                nc.vector.tensor_tensor(out=nwG, in0=nwG, in1=nyG, op=MULT)
                nc.vector.tensor_scalar(out=nwG, in0=nwG, scalar1=-0.5,
                                        scalar2=1.5, op0=MULT, op1=ADD)
                nc.vector.tensor_tensor(out=nwG, in0=nyG, in1=nwG, op=MULT)
                if not last:
                    for ci in range(CPG):
                        c = c0 + ci
                        if ci == CPG - 1:
                            nc.scalar.activation(
                                h_nat[:, c, :], upd[:, c, :], COPYF,
                                scale=nw_[:, c:c + 1])
                        else:
                            nc.vector.tensor_scalar_mul(
                                out=h_nat[:, c, :], in0=upd[:, c, :],
                                scalar1=nw_[:, c:c + 1])
                    pt = ps_t.tile([128, 2, CPG * 128], bf16, tag="pt")
                    for half in range(2):
                        for ci in range(CPG):
                            nc.tensor.transpose(
                                out=pt[:, half, ci * 128:(ci + 1) * 128],
                                in_=h_nat[:, c0 + ci,
                                          half * 128:(half + 1) * 128],
                                identity=ident[:])
                    nc.vector.tensor_copy(
                        out=hT[:, 0, c0 * 128:(c0 + CPG) * 128],
                        in_=pt[:, 0, :])
                    nc.scalar.copy(
                        out=hT[:, 1, c0 * 128:(c0 + CPG) * 128],
                        in_=pt[:, 1, :])
                else:
                    for ci in range(CPG):
                        c = c0 + ci
                        nc.vector.tensor_scalar_mul(
                            out=hf32[:, c, :], in0=upd[:, c, :],
                            scalar1=nw_[:, c:c + 1])

        if not SKIP_GATHER:
            # group 0 up front; later groups gather early / precompute
            # mid-group so the PSUM pool rotation never blocks step 0
            raws = {0: emit_gather(0)}
            emit_group_precompute(0, raws.pop(0))
            for g in range(GRP):
                for si in range(SPG):
                    s = g * SPG + si
                    if s >= NSTEPS:
                        continue
                    if si == 1 and g + 1 < GRP:
                        raws[g + 1] = emit_gather(g + 1)
                    if si == 4 and g + 1 < GRP:
                        emit_group_precompute(g + 1, raws.pop(g + 1))
                    scan_step(s)
        else:
            nc.vector.memset(encT[:], 0.0)
            nc.vector.memset(ew_all[:], 0.0)
            nc.vector.memset(ekm[:], 0.0)
            for s in range(NSTEPS):
                scan_step(s)

        if NSTEPS == 0:
            nc.vector.memset(hf32[:], 0.0)
        # ---- output: y[b, k, d] <- hf32[(k%4)*32+b, k//4, d] -------------
        y_main = y_d.ap()[:, 0:28, :].rearrange("b (kh kl) d -> b kl kh d",
                                                kl=4)
        for klo in range(4):
            nc.sync.dma_start(out=y_main[:, klo, :, :],
                              in_=hf32[klo * 32:(klo + 1) * 32, 0:7, :])
        nc.sync.dma_start(out=y_d.ap()[:, 28, :],
                          in_=hf32[0:32, 7, :])
        nc.sync.dma_start(out=y_d.ap()[:, 29, :],
                          in_=hf32[32:64, 7, :])

        for p in reversed(ctxs):
            p.__exit__(None, None, None)

    nc.compile()
    return nc


def _host_prep(prgrph, prgrph_mask, keys, embedding_matrix, U, V, W):
    """Build per-core input maps."""
    prg = np.asarray(prgrph).astype(np.int64)
    msk = np.asarray(prgrph_mask).astype(bool)
    keys = np.asarray(keys, dtype=np.float32)
    embm = np.asarray(embedding_matrix, dtype=np.float32)
    U = np.asarray(U, dtype=np.float32)
    V = np.asarray(V, dtype=np.float32)
    W = np.asarray(W, dtype=np.float32)

    emb_bf = embm.astype(_bf16)

    def halves(m):      # [256, 256] -> [128, 2, 256] bf16
        return np.ascontiguousarray(
            m.reshape(2, 128, D).swapaxes(0, 1).astype(_bf16))

    u_h, v_h, w_h = halves(U), halves(V), halves(W)

    ident = np.eye(128, dtype=_bf16)
    selsum = np.zeros((128, BL), dtype=_bf16)
    p_ar = np.arange(128)
    selsum[p_ar, p_ar % 32] = 1
    selkm = np.ascontiguousarray(selsum.T)
    mdiag = selsum.astype(np.float32)

    # token index layout: flat slot i=q*128+p, p=(l%4)*32+b, q=g*32+s_in*4+l//4
    tok = np.where(msk, prg, VOCAB).astype(np.int64)   # [B, S, L]
    sent_ok = msk.any(-1)                              # [B, S]

    in_maps = []
    for m in range(NCORES):
        b0 = m * BL
        t = tok[b0:b0 + BL]                            # [32, 64, 16]
        # quad dedup: one table row = the 4 l_hi embeddings of (b, s, l_lo)
        # quads[b, s, l_lo] = (t[b,s,l_lo], t[b,s,4+l_lo], t[b,s,8+l_lo], t[b,s,12+l_lo])
        quads = t.reshape(BL, S, 4, 4).transpose(0, 1, 3, 2)   # [b, s, l_lo, l_hi]
        qflat = np.ascontiguousarray(quads.reshape(-1, 4))
        uniq, inv = np.unique(qflat, axis=0, return_inverse=True)
        n_u = len(uniq)
        assert n_u <= 8192, f"unique quad overflow: {n_u}"
        emb_core = np.zeros((8192, 4, D), dtype=_bf16)
        safe = np.minimum(uniq, VOCAB)                  # VOCAB -> zero row
        ext = np.vstack([emb_bf, np.zeros((1, D), _bf16)])
        emb_core[:n_u] = ext[safe]
        emb_core = emb_core.reshape(8192, 4 * D)
        inv = inv.reshape(BL, S, 4)                     # [b, s, l_lo]
        # flat slot i = q*128 + p, p = l_lo*32 + b, q = s_in (per group)
        idx = np.zeros((128, 64), dtype=np.int16)       # [p, g*8+s_in]
        s_idx = np.arange(S)
        g_ar, si_ar = s_idx // SPG, s_idx % SPG
        for llo in range(4):
            p = llo * 32 + np.arange(BL)
            q = g_ar * 8 + si_ar
            idx[p[:, None], q[None, :]] = inv[:, :, llo].astype(np.int16)
        # wrap flat order i=q*128+p into [16, n/16] gather layout per group
        cols = []
        for g in range(GRP):
            flat = idx[:, g * 8:(g + 1) * 8].T.reshape(-1)   # i = s_in*128+p
            cols.append(flat.reshape(64, 16).T)
        idx16 = np.ascontiguousarray(np.tile(np.concatenate(cols, axis=1), (8, 1)))
        kT = np.zeros((D, J), dtype=_bf16)
        kloc = np.transpose(keys[b0:b0 + BL], (2, 1, 0))   # [D, K, BL]
        kT[:, :K * BL] = kloc.reshape(D, K * BL)[:, :]
        # j = k*32 + b -> reshape (K, BL) row-major matches k*32+b
        keysT_h = np.ascontiguousarray(kT.reshape(2, 128, J).swapaxes(0, 1))
        bias = np.zeros((128, S), dtype=np.float32)
        ok = sent_ok[b0:b0 + BL]                       # [32, 64]
        bias[:, :] = np.where(ok, 0.0, NEG)[np.arange(128) % 32, :]
        in_maps.append({
            "emb": emb_core, "idx": idx16, "keysT": keysT_h,
            "u": u_h, "v": v_h, "w": w_h, "bias": bias,
            "selsum": selsum, "selkm": selkm, "mdiag": mdiag,
            "ident": ident,
        })
    return in_maps


def _patch_ldw_opt():
    # flip walrus's --enable-ldw-opt for our own compile invocation:
    # gate-mm/hU-mm pairs share lhsT, so merging redundant LDWEIGHTS
    # saves PE issue slots
    import concourse.bass_utils as _bu
    if getattr(_bu, "_bk_ldw_patched", False):
        return
    _orig_rc = _bu.run_command

    def _rc(argv, **kw):
        argv = ["--enable-ldw-opt=true" if a == "--enable-ldw-opt=false"
                else a for a in argv]
        return _orig_rc(argv, **kw)

    _bu.run_command = _rc
    _bu._bk_ldw_patched = True


def kernel(**inputs):
    global LAST_EXEC_NS
    from concourse.bass_utils import run_bass_kernel_spmd
    if os.environ.get("BK_LDW_OPT", "0") == "1":
        _patch_ldw_opt()

    trace = os.environ.get("BASS_KERNEL_TRACE", "0") == "1"
    if trace:
        try:
            import sys, types, contextlib

            if "antenv.axon_hooks" not in sys.modules:
                mod = types.ModuleType("antenv.axon_hooks")
                _h = [None]
                mod.set_axon_ntff_profile_hook = lambda h: _h.__setitem__(0, h)
                mod.get_axon_ntff_profile_hook = lambda: _h[0]
                sys.modules["antenv.axon_hooks"] = mod
                import antenv
                antenv.axon_hooks = mod
                from trn_agent_boot.trn_boot import _ntff_profile_via_ctypes
                mod.set_axon_ntff_profile_hook(
                    _ntff_profile_via_ctypes("/opt/axon/libaxon_pjrt.so"))
        except Exception as e:
            print("trace hook unavailable:", e)
            trace = False

    nc = _build_nc()
    in_maps = _host_prep(**inputs)
    res = run_bass_kernel_spmd(nc, in_maps, list(range(NCORES)), trace=trace)
    if trace:
        LAST_EXEC_NS = res.exec_time_ns
    out = np.concatenate([res.results[m]["y"] for m in range(NCORES)], axis=0)
    return out.astype(np.float32)

